# revision 51
# baseline (speedup 1.0000x reference)
"""SAGEConv (mean aggregation) + ReLU on 8 Trainium2 NeuronCores.

Problem: h = relu(mean_agg(x, edges) @ W_l.T + b_l + x @ W_r.T)
  x [8, 55296, 64] f32, 221184 random edges, W [256, 64].

Strategy (dst-sharded, all-batch, v2):
  Core c owns destination nodes [c*6912, (c+1)*6912) for ALL 8 batches.
  x is re-laid host-side as node-major rows of 512 (8 batches x 64 feats),
  cast to bf16, split into lo/hi halves (int16 gather-index limit).
  Destinations are processed in 14 superblocks (13x512 + 1x256 dsts); each
  superblock has 4 (resp. 2) windows of 128 dsts.
  Per superblock:
    - dma_gather fetches per-edge source rows (1024B) in (window, round)
      order -> edge-major msgs columns [128 edges, 512 feats] bf16.
      SPMD-pad columns use trailing -1 indices (skipped, no HBM traffic).
    - Selection matrices S[e, d] = (dstloc[e] == d) built on DVE in one
      bulk is_equal per superblock; TensorE accumulates
      aggT[feat, dst] += msgs_col^T @ S_col into a 4-bank PSUM tile
      [128, 4(fc), 512] (fc = batch-pair), N=128 per matmul.
    - agg evacuation: DVE multiplies by 1/deg (deg-0 dsts masked to 0) and
      writes bf16 comb tiles [feat, dst]; the self/root x rows arrive by
      plain DMA from host-transposed tensors directly into the other
      partition half of comb (batch-parity interleaved) -- no PE transposes.
    - Phase B: stacked [W_l;W_r] (parity-swapped for odd batches) is the
      stationary operand; comb streams through at N=sbsz producing
      hT[hid, dst] in PSUM; relu+bf16-cast on DVE/ACT; one output DMA per
      superblock writes outT [8, 2, 128, 6912] bf16.
  Host: outT -> [8, 55296, 256] f32 transpose/upcast + concat cores.
"""

import numpy as np

N_NODES = 55296
F_IN = 64
F_HID = 256
BATCH = 8
NCORE = 8
ND = N_NODES // NCORE          # 6912 dsts per core
HALF = N_NODES // 2            # 27648
EW = BATCH * F_IN              # 512 elems per node row
WSZ = 128                      # aggregation window (dsts per matmul)
SB_FULL = 512                  # dsts per superblock
NSB = 14                       # 13x512 + 1x256
SBS = [SB_FULL] * 13 + [ND - 13 * SB_FULL]
SBB = [i * SB_FULL for i in range(NSB)]
MSGS_BUFS = 6
MSGS_FP8 = True                # gather/aggregate messages in fp8e4m3
ZROW = HALF                    # index of the appended all-zero row

_cache = {}


def _build(schedule, has_bias):
    import concourse.bacc as bacc
    import concourse.tile as tile
    import concourse.mybir as mybir
    from concourse.library_config import mlp

    # schedule: tuple over 28 calls (sb-major, half-minor) of tuples of
    # winbases (one per column slot, in slot order)
    bf16 = mybir.dt.bfloat16
    f32 = mybir.dt.float32
    mdt = mybir.dt.float8e4 if MSGS_FP8 else bf16

    calls = [(schedule[2 * k], schedule[2 * k + 1]) for k in range(NSB)]
    sb_ncols = [len(a) + len(b) for a, b in calls]
    tot_cols = sum(sb_ncols)
    max_sb_cols = max(sb_ncols)
    tot_idx = tot_cols * 128

    nc = bacc.Bacc(None, target_bir_lowering=False, debug=False,
                   num_swdge_queues=4)
    with tile.TileContext(nc) as tc:
        with tc.tile_pool(name="dram", bufs=1, space="DRAM") as dram:
            xab_lo = dram.tile([HALF + 1, EW], mdt, kind="ExternalInput")
            xab_hi = dram.tile([HALF + 1, EW], mdt, kind="ExternalInput")
            # superblock-major layouts so each per-sb DMA moves large
            # contiguous per-partition runs (4KB / 16KB descriptors)
            xsT_ev = dram.tile([NSB, F_IN, 4, SB_FULL], bf16,
                               kind="ExternalInput")
            xsT_od = dram.tile([NSB, F_IN, 4, SB_FULL], bf16,
                               kind="ExternalInput")
            gidx = dram.tile([128, tot_idx // 16], mybir.dt.int16,
                             kind="ExternalInput")
            dstloc = dram.tile([128, tot_cols], bf16, kind="ExternalInput")
            iota_rep = dram.tile([128, WSZ], bf16, kind="ExternalInput")
            invdeg_rep = dram.tile([128, ND], bf16, kind="ExternalInput")
            w_ev = dram.tile([128, F_HID], bf16, kind="ExternalInput")
            w_od = dram.tile([128, F_HID], bf16, kind="ExternalInput")
            zeros_d = dram.tile([128, SB_FULL], bf16, kind="ExternalInput")
            vreg_d = dram.tile([128, NSB * 2], mybir.dt.int32,
                               kind="ExternalInput")
            if has_bias:
                bias_d = dram.tile([128, 2], f32, kind="ExternalInput")
            out = dram.tile([NSB, 128, BATCH, 2, SB_FULL], bf16,
                            kind="ExternalOutput")

            with (
                tc.tile_pool(name="const", bufs=1) as constp,
                tc.tile_pool(name="msgs", bufs=MSGS_BUFS) as msgsp,
                tc.tile_pool(name="spool", bufs=3) as spool,
                tc.tile_pool(name="comb", bufs=2) as combp,
                tc.tile_pool(name="hsb", bufs=2) as hsbp,
                tc.tile_pool(name="aggps", bufs=1, space="PSUM") as aggpsp,
                tc.tile_pool(name="hps", bufs=2, space="PSUM") as hpsp,
            ):
                nc.gpsimd.load_library(mlp)

                # gather-critical consts first so the first gathers launch
                # as early as possible
                vreg_t = constp.tile([128, NSB * 2], mybir.dt.int32)
                nc.sync.dma_start(out=vreg_t[:], in_=vreg_d[:])
                gidx_t = constp.tile([128, tot_idx // 16], mybir.dt.int16)
                nc.sync.dma_start(out=gidx_t[:], in_=gidx[:])
                zeros_t = constp.tile([128, SB_FULL], bf16)
                nc.sync.dma_start(out=zeros_t[:], in_=zeros_d[:])
                dstloc_t = constp.tile([128, tot_cols], bf16)
                nc.sync.dma_start(out=dstloc_t[:], in_=dstloc[:])
                iota_t = constp.tile([128, WSZ], bf16)
                nc.sync.dma_start(out=iota_t[:], in_=iota_rep[:])
                w_ev_t = constp.tile([128, F_HID], bf16)
                nc.sync.dma_start(out=w_ev_t[:], in_=w_ev[:])
                w_od_t = constp.tile([128, F_HID], bf16)
                nc.sync.dma_start(out=w_od_t[:], in_=w_od[:])
                invdeg_t = constp.tile([128, ND], bf16)
                nc.sync.dma_start(out=invdeg_t[:], in_=invdeg_rep[:])
                # one register per gather call; load a few upfront, the rest
                # rolling (two superblocks ahead) so the preload burst does
                # not delay the first gathers
                vrs = [nc.gpsimd.alloc_register(f"gather_v{i}")
                       for i in range(NSB * 2)]
                for ci in range(6):
                    nc.gpsimd.reg_load(vrs[ci], vreg_t[0:1, ci:ci + 1])
                gq = 0  # SWDGE queue round-robin: queue q runs on Q7
                # cores {2q, 2q+1}, so 4 queues generate concurrently
                if has_bias:
                    bias_t = constp.tile([128, 2], f32)
                    nc.sync.dma_start(out=bias_t[:], in_=bias_d[:])

                # one-time zero fill of the agg banks so that elements never
                # touched by any matmul hold 0.0 (not raw PSUM garbage)
                agg0 = aggpsp.tile([128, 4, SB_FULL], f32, tag="agg")
                for fc in range(4):
                    nc.tensor.matmul(
                        out=agg0[:, fc, :],
                        lhsT=zeros_t[:, 0:128],
                        rhs=zeros_t[:],
                        start=True, stop=True)
                # one-time zero fill of the msgs buffers: slots skipped by
                # the per-core gather trim must hold finite (non-NaN) bits
                for mi in range(MSGS_BUFS):
                    mz = msgsp.tile([128, max_sb_cols * EW], mdt,
                                    tag="msgs")
                    if mi % 2:
                        nc.vector.memset(mz[:], 0)
                    else:
                        nc.scalar.activation(
                            out=mz[:].bitcast(mybir.dt.bfloat16),
                            in_=zeros_t[:, 0:1].to_broadcast(
                                [128, max_sb_cols * EW // 2]),
                            func=mybir.ActivationFunctionType.Copy)

                idx_off = 0
                col_off = 0
                relu_flip = 0
                prev = None  # (comb_t, sbsz, k) pending phase B

                for k in range(NSB):
                    sbsz = SBS[k]
                    wbA, wbB = calls[k]
                    ncA, ncB = len(wbA), len(wbB)
                    ncols = ncA + ncB
                    for ci in range(2 * k + 6, min(2 * k + 8, NSB * 2)):
                        nc.gpsimd.reg_load(vrs[ci], vreg_t[0:1, ci:ci + 1])
                    m_t = msgsp.tile([128, max_sb_cols * EW], mdt,
                                     tag="msgs")
                    m3 = m_t[:].rearrange("p (c e) -> p c e", e=EW)
                    for (xsrc, c0, cn, ci) in (
                            (xab_lo, 0, ncA, 2 * k),
                            (xab_hi, ncA, ncB, 2 * k + 1)):
                        if cn == 0:
                            continue
                        nidx = cn * 128
                        vr = vrs[ci]
                        nc.gpsimd.dma_gather(
                            out_ap=m3[:, c0:c0 + cn, :],
                            in_ap=xsrc[:],
                            idxs_ap=gidx_t[:, idx_off // 16:
                                           (idx_off + nidx) // 16],
                            num_idxs=nidx,
                            num_idxs_reg=vr,
                            elem_size=EW,
                            single_packet=False,
                            queue_num=gq,
                        )
                        gq = (gq + 1) % 4
                        idx_off += nidx

                    # bulk selection build: S[p, c, d] = (dstloc[p,c] == d)
                    s_t = spool.tile([128, max_sb_cols, WSZ], mdt,
                                     tag="sel")
                    try:
                        nc.vector.tensor_tensor(
                            out=s_t[:, 0:ncols, :],
                            in0=iota_t[:].unsqueeze(1).to_broadcast(
                                [128, ncols, WSZ]),
                            in1=dstloc_t[:, col_off:col_off + ncols]
                            .unsqueeze(2).to_broadcast([128, ncols, WSZ]),
                            op=mybir.AluOpType.is_equal,
                        )
                    except Exception:
                        for ci in range(ncols):
                            nc.vector.tensor_tensor(
                                out=s_t[:, ci, :],
                                in0=iota_t[:],
                                in1=dstloc_t[:, col_off + ci:col_off + ci + 1]
                                .to_broadcast([128, WSZ]),
                                op=mybir.AluOpType.is_equal,
                            )

                    agg_t = aggpsp.tile([128, 4, SB_FULL], f32, tag="agg")
                    wbs = list(wbA) + list(wbB)
                    for ci, wb in enumerate(wbs):
                        for fc in range(4):
                            nc.tensor.matmul(
                                out=agg_t[:, fc, wb:wb + WSZ],
                                lhsT=m3[:, ci, fc * 128:(fc + 1) * 128],
                                rhs=s_t[:, ci, :],
                                start=(ci == 0),
                                stop=(ci == len(wbs) - 1),
                            )

                    # comb assembly: agg x invdeg (DVE) + self-x rows (DMA)
                    comb_t = combp.tile([128, 8, SB_FULL], bf16, tag="comb")
                    sbb = SBB[k]
                    ivd = invdeg_t[:, sbb:sbb + sbsz]
                    comb4 = comb_t[:].rearrange(
                        "p (f two) d -> p f two d", two=2)
                    # even batches live in partitions 0:64 of agg, odd in
                    # 64:128; scale all 4 fc in one op per parity
                    nc.vector.tensor_mul(
                        out=comb4[0:64, :, 0, 0:sbsz],
                        in0=agg_t[0:64, :, 0:sbsz],
                        in1=ivd[0:64, :].unsqueeze(1)
                        .to_broadcast([64, 4, sbsz]))
                    nc.vector.tensor_mul(
                        out=comb4[64:128, :, 1, 0:sbsz],
                        in0=agg_t[64:128, :, 0:sbsz],
                        in1=ivd[64:128, :].unsqueeze(1)
                        .to_broadcast([64, 4, sbsz]))
                    # x of even batches -> partitions 64:128 of even slots
                    nc.sync.dma_start(
                        out=comb4[64:128, :, 0, 0:sbsz],
                        in_=xsT_ev[k, :, :, 0:sbsz],
                    )
                    # x of odd batches -> partitions 0:64 of odd slots
                    nc.sync.dma_start(
                        out=comb4[0:64, :, 1, 0:sbsz],
                        in_=xsT_od[k, :, :, 0:sbsz],
                    )

                    col_off += ncols

                    # phase B for the PREVIOUS superblock (software pipeline
                    # so PE has work while DVE drains agg of this one)
                    if prev is not None:
                        relu_flip = _phase_b(
                            nc, mybir, tc, hpsp, hsbp, out,
                            w_ev_t, w_od_t,
                            bias_t if has_bias else None,
                            prev, relu_flip)
                    prev = (comb_t, sbsz, k)

                relu_flip = _phase_b(
                    nc, mybir, tc, hpsp, hsbp, out, w_ev_t, w_od_t,
                    bias_t if has_bias else None, prev, relu_flip)

    nc.compile()
    names = dict(
        xab_lo=xab_lo.name, xab_hi=xab_hi.name,
        xsT_ev=xsT_ev.name, xsT_od=xsT_od.name,
        gidx=gidx.name, dstloc=dstloc.name,
        iota_rep=iota_rep.name, invdeg_rep=invdeg_rep.name,
        w_ev=w_ev.name, w_od=w_od.name, zeros_d=zeros_d.name,
        vreg=vreg_d.name, out=out.name,
        bias_d=(bias_d.name if has_bias else None),
    )
    return nc, names


def _phase_b(nc, mybir, tc, hpsp, hsbp, out, w_ev_t, w_od_t, bias_t,
             prev, relu_flip):
    f32 = mybir.dt.float32
    bf16 = mybir.dt.bfloat16
    comb_t, sbsz, k = prev
    sbb = SBB[k]
    hsb_t = hsbp.tile([128, 16, SB_FULL], bf16, tag="hsb")
    # hsb slot index = b*2 + hh = 4*fc + 2*par + hh
    hsb4 = hsb_t[:, :, 0:sbsz].rearrange(
        "p (b2 four) d -> p b2 four d", four=4)
    for hh in range(2):
        for par in range(2):
            w_t = w_od_t if par else w_ev_t
            for fc in range(4):
                b = 2 * fc + par
                pairpos = fc % 2
                if pairpos == 0:
                    h_ps = hpsp.tile([128, 2, SB_FULL], f32, tag="hps")
                nc.tensor.matmul(
                    out=h_ps[:, pairpos, 0:sbsz],
                    lhsT=w_t[:, hh * 128:(hh + 1) * 128],
                    rhs=comb_t[:, b, 0:sbsz],
                    start=True, stop=True,
                )
                if bias_t is not None:
                    nc.vector.tensor_add(
                        out=h_ps[:, pairpos, 0:sbsz],
                        in0=h_ps[:, pairpos, 0:sbsz],
                        in1=bias_t[:, hh:hh + 1].to_broadcast([128, sbsz]))
                if pairpos == 1:
                    # relu+cast both fc of the pair tile in one op
                    dst = hsb4[:, fc - 1:fc + 1, 2 * par + hh, :]
                    if relu_flip % 8 == 0:
                        nc.vector.tensor_relu(
                            out=dst, in_=h_ps[:, :, 0:sbsz])
                    else:
                        nc.scalar.activation(
                            out=dst, in_=h_ps[:, :, 0:sbsz],
                            func=mybir.ActivationFunctionType.Relu)
                    relu_flip += 1
    nc.sync.dma_start(
        out=out[k, :, :, :, 0:sbsz],
        in_=hsb_t[:, :, 0:sbsz].rearrange("p (b h) d -> p b h d", h=2),
    )
    return relu_flip


def _prep(x, edge_src, edge_dst, W_l, b_l, W_r):
    from ml_dtypes import bfloat16

    deg = np.bincount(edge_dst, minlength=N_NODES).astype(np.float32)
    invdeg = (1.0 / np.maximum(deg, 1.0)).astype(np.float32)
    invdeg[deg == 0] = 0.0  # mask never-written agg lanes

    if MSGS_FP8:
        from ml_dtypes import float8_e4m3 as mdt_np
    else:
        mdt_np = bfloat16
    xn = np.ascontiguousarray(x.transpose(1, 0, 2)).reshape(N_NODES, EW)
    xn_m = xn.astype(mdt_np)
    zrow = np.zeros((1, EW), dtype=mdt_np)
    xab_lo = np.ascontiguousarray(np.vstack([xn_m[:HALF], zrow]))
    xab_hi = np.ascontiguousarray(np.vstack([xn_m[HALF:], zrow]))

    iota_rep = np.broadcast_to(
        np.arange(WSZ, dtype=np.float32).astype(bfloat16)[None, :],
        (128, WSZ)).copy()
    WlT = W_l.T.astype(np.float32)
    WrT = W_r.T.astype(np.float32)
    w_ev = np.vstack([WlT, WrT]).astype(bfloat16)
    w_od = np.vstack([WrT, WlT]).astype(bfloat16)
    zeros_d = np.zeros((128, SB_FULL), dtype=bfloat16)
    has_bias = bool(np.any(b_l != 0))
    bias_d = (np.stack([b_l[:128], b_l[128:]], axis=1).astype(np.float32)
              if has_bias else None)

    core = edge_dst // ND
    # per (core, sb, half, window): sorted edge lists
    percore = []
    NW = [sz // WSZ for sz in SBS]  # windows per sb: 4,...,4,2
    cnt = np.zeros((NCORE, NSB, 2, 4), np.int64)
    for c in range(NCORE):
        sel = core == c
        ed = (edge_dst[sel] - c * ND).astype(np.int64)
        es = edge_src[sel].astype(np.int64)
        sb = np.minimum(ed // SB_FULL, NSB - 1)
        h = (es >= HALF).astype(np.int64)
        w = (ed - sb * SB_FULL) // WSZ
        order = np.lexsort((es, ed, w, h, sb))
        ed, es, sb, h, w = (a[order] for a in (ed, es, sb, h, w))
        key = ((sb * 2 + h) * 4 + w)
        bounds = np.searchsorted(key, np.arange(NSB * 2 * 4 + 1))
        for sbi in range(NSB):
            for hh in range(2):
                for wi in range(NW[sbi]):
                    kk = (sbi * 2 + hh) * 4 + wi
                    n = bounds[kk + 1] - bounds[kk]
                    cnt[c, sbi, hh, wi] = -(-n // 128)
        percore.append((ed, es, bounds))

    K = cnt.max(axis=0)  # [NSB, 2, 4] column slots per window

    # slot order per (sb, half): by cross-core usage count descending, so
    # each core's real edges cluster at the front of the call and the
    # trailing -1 trim (zero HBM cost) removes most SPMD padding
    winbase_lists = []
    slotmaps = []  # per (sb, half): list of (window, round)
    for sbi in range(NSB):
        for hh in range(2):
            slots = []
            maxr = int(K[sbi, hh, :NW[sbi]].max()) if NW[sbi] else 0
            for r in range(maxr):
                for wi in range(NW[sbi]):
                    if K[sbi, hh, wi] > r:
                        usage = int((cnt[:, sbi, hh, wi] > r).sum())
                        slots.append((-usage, r, wi))
            slots = [(wi, r) for _, r, wi in sorted(slots)]
            slotmaps.append(slots)
            winbase_lists.append(tuple(wi * WSZ for wi, r in slots))

    tot_cols = sum(len(s) for s in slotmaps)

    max_sb_cols = max(
        len(slotmaps[2 * k]) + len(slotmaps[2 * k + 1]) for k in range(NSB))

    # Build per-(call, core) slot index/dst arrays, then a SHARED valid
    # count V per call: num_idxs_reg is baked into the SPMD instruction so
    # every core must have exactly V non-negative indices (trailing -1s
    # are skipped by the gather ucode at zero HBM cost). A -1 slot exposes
    # stale SBUF, which is only safe if a previous superblock on the same
    # msgs buffer gathered that slot (coverage is uniform across cores
    # because all cores gather exactly [0, V)).
    call_sidx = [[None] * NCORE for _ in range(NSB * 2)]
    for c in range(NCORE):
        ed, es, bounds = percore[c]
        for sbi in range(NSB):
            for hh in range(2):
                slots = slotmaps[sbi * 2 + hh]
                nslots = len(slots)
                sidx = np.full((nslots, 128), -1, np.int32)
                for si, (wi, r) in enumerate(slots):
                    kk = (sbi * 2 + hh) * 4 + wi
                    lo, hi = bounds[kk], bounds[kk + 1]
                    e0 = lo + r * 128
                    e1 = min(lo + (r + 1) * 128, hi)
                    if e1 > e0:
                        n = e1 - e0
                        sidx[si, :n] = es[e0:e1] - hh * HALF
                call_sidx[sbi * 2 + hh][c] = sidx

    # per-core dstloc (separate pass, simple)
    dl_cores = np.full((NCORE, tot_cols, 128), -1.0, np.float32)
    for c in range(NCORE):
        ed, es, bounds = percore[c]
        ci_base = 0
        for sbi in range(NSB):
            for hh in range(2):
                slots = slotmaps[sbi * 2 + hh]
                for si, (wi, r) in enumerate(slots):
                    kk = (sbi * 2 + hh) * 4 + wi
                    lo, hi = bounds[kk], bounds[kk + 1]
                    e0 = lo + r * 128
                    e1 = min(lo + (r + 1) * 128, hi)
                    if e1 > e0:
                        dl_cores[c, ci_base + si, :e1 - e0] = (
                            ed[e0:e1] - SBB[sbi] - wi * WSZ)
                ci_base += len(slots)

    # per-core valid count V per call (num_idxs_reg is loaded from data at
    # runtime). The msgs buffers are memset to zero once at kernel start,
    # so slots beyond V expose only finite stale data (killed by S=0).
    schedule = tuple(winbase_lists)
    Vs = np.zeros((NCORE, NSB * 2), np.int32)
    for c in range(NCORE):
        for ci in range(NSB * 2):
            nidx = len(slotmaps[ci]) * 128
            if nidx == 0:
                continue
            flat = call_sidx[ci][c].reshape(-1)
            nz = np.nonzero(flat >= 0)[0]
            V = int(nz[-1]) + 1 if len(nz) else 0
            Vs[c, ci] = min(max(V, 16), nidx)

    in_maps = []
    for c in range(NCORE):
        gidx_chunks = []
        for ci in range(NSB * 2):
            nslots = len(slotmaps[ci])
            if nslots == 0:
                continue
            flat = call_sidx[ci][c].reshape(-1).copy()
            V = int(Vs[c, ci])
            head = flat[:V]
            head[head < 0] = ZROW
            flat[V:] = -1
            sl = flat.astype(np.int16)
            gidx_chunks.append(np.tile(sl.reshape(-1, 16).T, (8, 1)))
        gidx_arr = np.ascontiguousarray(np.concatenate(gidx_chunks, axis=1))
        dstloc = np.ascontiguousarray(dl_cores[c].T.astype(bfloat16))
        vreg_c = np.broadcast_to(Vs[c][None, :], (128, NSB * 2)).copy()

        invdeg_c = np.broadcast_to(
            invdeg[c * ND:(c + 1) * ND].astype(bfloat16)[None, :],
            (128, ND)).copy()
        xc = x[:, c * ND:(c + 1) * ND, :]  # [8, ND, 64]
        # [NSB, F_IN, 4, SB_FULL] superblock-major, zero-padded last sb
        xsT_ev = np.zeros((NSB, F_IN, 4, SB_FULL), bfloat16)
        xsT_od = np.zeros((NSB, F_IN, 4, SB_FULL), bfloat16)
        for k in range(NSB):
            sl = xc[:, SBB[k]:SBB[k] + SBS[k], :]  # [8, sbsz, 64]
            xsT_ev[k, :, :, :SBS[k]] = sl[0::2].transpose(2, 0, 1)
            xsT_od[k, :, :, :SBS[k]] = sl[1::2].transpose(2, 0, 1)

        in_maps.append(dict(
            xab_lo=xab_lo, xab_hi=xab_hi, xsT_ev=xsT_ev, xsT_od=xsT_od,
            gidx=gidx_arr, dstloc=dstloc, iota_rep=iota_rep,
            invdeg_rep=invdeg_c, w_ev=w_ev, w_od=w_od, zeros_d=zeros_d,
            vreg=vreg_c, bias_d=bias_d,
        ))
    return schedule, has_bias, in_maps


def kernel(x, edge_src, edge_dst, W_l, b_l, W_r):
    from concourse.bass_utils import run_bass_kernel_spmd

    x = np.asarray(x, dtype=np.float32)
    edge_src = np.asarray(edge_src, dtype=np.int32)
    edge_dst = np.asarray(edge_dst, dtype=np.int32)
    W_l = np.asarray(W_l, dtype=np.float32)
    b_l = np.asarray(b_l, dtype=np.float32)
    W_r = np.asarray(W_r, dtype=np.float32)

    schedule, has_bias, in_maps = _prep(x, edge_src, edge_dst, W_l, b_l, W_r)
    key = (schedule, has_bias)
    if key not in _cache:
        _cache[key] = _build(schedule, has_bias)
    nc, names = _cache[key]

    run_maps = []
    for m in in_maps:
        rm = {names[k]: v for k, v in m.items()
              if names.get(k) is not None and v is not None}
        run_maps.append(rm)
    res = run_bass_kernel_spmd(nc, run_maps, list(range(NCORE)))
    outs = []
    for c in range(NCORE):
        oc = np.asarray(res.results[c][names["out"]]).astype(np.float32)
        # [NSB, 128 hid, 8, 2, SB_FULL] -> [8, ND, 256]
        full = oc.transpose(2, 0, 4, 3, 1).reshape(
            BATCH, NSB * SB_FULL, F_HID)
        parts = [full[:, k * SB_FULL:k * SB_FULL + SBS[k]]
                 for k in range(NSB)]
        outs.append(np.concatenate(parts, axis=1))
    return np.concatenate(outs, axis=1)


# revision 57
# speedup vs baseline: 1.1013x; 1.1013x over previous
"""SAGEConv (mean aggregation) + ReLU on 8 Trainium2 NeuronCores.

Problem: h = relu(mean_agg(x, edges) @ W_l.T + b_l + x @ W_r.T)
  x [8, 55296, 64] f32, 221184 random edges, W [256, 64].

Strategy (dst-sharded, all-batch, v2):
  Core c owns destination nodes [c*6912, (c+1)*6912) for ALL 8 batches.
  x is re-laid host-side as node-major rows of 512 (8 batches x 64 feats),
  cast to bf16, split into lo/hi halves (int16 gather-index limit).
  Destinations are processed in 14 superblocks (13x512 + 1x256 dsts); each
  superblock has 4 (resp. 2) windows of 128 dsts.
  Per superblock:
    - dma_gather fetches per-edge source rows (1024B) in (window, round)
      order -> edge-major msgs columns [128 edges, 512 feats] bf16.
      SPMD-pad columns use trailing -1 indices (skipped, no HBM traffic).
    - Selection matrices S[e, d] = (dstloc[e] == d) built on DVE in one
      bulk is_equal per superblock; TensorE accumulates
      aggT[feat, dst] += msgs_col^T @ S_col into a 4-bank PSUM tile
      [128, 4(fc), 512] (fc = batch-pair), N=128 per matmul.
    - agg evacuation: DVE multiplies by 1/deg (deg-0 dsts masked to 0) and
      writes bf16 comb tiles [feat, dst]; the self/root x rows arrive by
      plain DMA from host-transposed tensors directly into the other
      partition half of comb (batch-parity interleaved) -- no PE transposes.
    - Phase B: stacked [W_l;W_r] (parity-swapped for odd batches) is the
      stationary operand; comb streams through at N=sbsz producing
      hT[hid, dst] in PSUM; relu+bf16-cast on DVE/ACT; one output DMA per
      superblock writes outT [8, 2, 128, 6912] bf16.
  Host: outT -> [8, 55296, 256] f32 transpose/upcast + concat cores.
"""

import numpy as np

N_NODES = 55296
F_IN = 64
F_HID = 256
BATCH = 8
NCORE = 8
ND = N_NODES // NCORE          # 6912 dsts per core
HALF = N_NODES // 2            # 27648
EW = BATCH * F_IN              # 512 elems per node row
WSZ = 128                      # aggregation window (dsts per matmul)
SB_FULL = 512                  # dsts per superblock
NSB = 14                       # 13x512 + 1x256
SBS = [SB_FULL] * 13 + [ND - 13 * SB_FULL]
SBB = [i * SB_FULL for i in range(NSB)]
MSGS_BUFS = 6
MSGS_FP8 = True                # gather/aggregate messages in fp8e4m3
ZROW = HALF                    # index of the appended all-zero row

_cache = {}


def _build(schedule, has_bias):
    import concourse.bacc as bacc
    import concourse.tile as tile
    import concourse.mybir as mybir
    from concourse.library_config import mlp

    # schedule: tuple over 28 calls (sb-major, half-minor) of tuples of
    # winbases (one per column slot, in slot order)
    bf16 = mybir.dt.bfloat16
    f32 = mybir.dt.float32
    mdt = mybir.dt.float8e4 if MSGS_FP8 else bf16

    calls = [(schedule[2 * k], schedule[2 * k + 1]) for k in range(NSB)]
    sb_ncols = [len(a) + len(b) for a, b in calls]
    tot_cols = sum(sb_ncols)
    max_sb_cols = max(sb_ncols)
    tot_idx = tot_cols * 128

    nc = bacc.Bacc(None, target_bir_lowering=False, debug=False,
                   num_swdge_queues=4)
    with tile.TileContext(nc) as tc:
        with tc.tile_pool(name="dram", bufs=1, space="DRAM") as dram:
            xab_lo = dram.tile([HALF + 1, EW], mdt, kind="ExternalInput")
            xab_hi = dram.tile([HALF + 1, EW], mdt, kind="ExternalInput")
            # superblock-major layouts so each per-sb DMA moves large
            # contiguous per-partition runs (4KB / 16KB descriptors)
            xsT_ev = dram.tile([NSB, F_IN, 4, SB_FULL], bf16,
                               kind="ExternalInput")
            xsT_od = dram.tile([NSB, F_IN, 4, SB_FULL], bf16,
                               kind="ExternalInput")
            gidx = dram.tile([128, tot_idx // 16], mybir.dt.int16,
                             kind="ExternalInput")
            dstloc = dram.tile([128, tot_cols], bf16, kind="ExternalInput")
            iota_rep = dram.tile([128, WSZ], bf16, kind="ExternalInput")
            invdeg_rep = dram.tile([128, ND], bf16, kind="ExternalInput")
            w_ev = dram.tile([128, F_HID], bf16, kind="ExternalInput")
            w_od = dram.tile([128, F_HID], bf16, kind="ExternalInput")
            zeros_d = dram.tile([128, SB_FULL], bf16, kind="ExternalInput")
            vreg_d = dram.tile([128, NSB * 4], mybir.dt.int32,
                               kind="ExternalInput")
            if has_bias:
                bias_d = dram.tile([128, 2], f32, kind="ExternalInput")
            out = dram.tile([NSB, 128, BATCH, 2, SB_FULL], bf16,
                            kind="ExternalOutput")

            with (
                tc.tile_pool(name="const", bufs=1) as constp,
                tc.tile_pool(name="msgs", bufs=MSGS_BUFS) as msgsp,
                tc.tile_pool(name="spool", bufs=3) as spool,
                tc.tile_pool(name="comb", bufs=2) as combp,
                tc.tile_pool(name="hsb", bufs=2) as hsbp,
                tc.tile_pool(name="aggps", bufs=1, space="PSUM") as aggpsp,
                tc.tile_pool(name="hps", bufs=2, space="PSUM") as hpsp,
            ):
                nc.gpsimd.load_library(mlp)

                # gather-critical consts first so the first gathers launch
                # as early as possible
                vreg_t = constp.tile([128, NSB * 4], mybir.dt.int32)
                nc.sync.dma_start(out=vreg_t[:], in_=vreg_d[:])
                gidx_t = constp.tile([128, tot_idx // 16], mybir.dt.int16)
                nc.sync.dma_start(out=gidx_t[:], in_=gidx[:])
                zeros_t = constp.tile([128, SB_FULL], bf16)
                nc.sync.dma_start(out=zeros_t[:], in_=zeros_d[:])
                dstloc_t = constp.tile([128, tot_cols], bf16)
                nc.sync.dma_start(out=dstloc_t[:], in_=dstloc[:])
                iota_t = constp.tile([128, WSZ], bf16)
                nc.sync.dma_start(out=iota_t[:], in_=iota_rep[:])
                w_ev_t = constp.tile([128, F_HID], bf16)
                nc.sync.dma_start(out=w_ev_t[:], in_=w_ev[:])
                w_od_t = constp.tile([128, F_HID], bf16)
                nc.sync.dma_start(out=w_od_t[:], in_=w_od[:])
                invdeg_t = constp.tile([128, ND], bf16)
                nc.sync.dma_start(out=invdeg_t[:], in_=invdeg_rep[:])
                # one register per gather call; load a few upfront, the rest
                # rolling (two superblocks ahead) so the preload burst does
                # not delay the first gathers
                NVR = 32
                vrs = [nc.gpsimd.alloc_register(f"gather_v{i}")
                       for i in range(NVR)]
                for si in range(12):
                    nc.gpsimd.reg_load(vrs[si % NVR],
                                       vreg_t[0:1, si:si + 1])
                gq = 0  # SWDGE queue round-robin: queue q runs on Q7
                # cores {2q, 2q+1}, so 4 queues generate concurrently
                if has_bias:
                    bias_t = constp.tile([128, 2], f32)
                    nc.sync.dma_start(out=bias_t[:], in_=bias_d[:])

                # one-time zero fill of the agg banks so that elements never
                # touched by any matmul hold 0.0 (not raw PSUM garbage)
                agg0 = aggpsp.tile([128, 4, SB_FULL], f32, tag="agg")
                for fc in range(4):
                    nc.tensor.matmul(
                        out=agg0[:, fc, :],
                        lhsT=zeros_t[:, 0:128],
                        rhs=zeros_t[:],
                        start=True, stop=True)
                # one-time zero fill of the msgs buffers: slots skipped by
                # the per-core gather trim must hold finite (non-NaN) bits
                for mi in range(MSGS_BUFS):
                    mz = msgsp.tile([128, max_sb_cols * EW], mdt,
                                    tag="msgs")
                    if mi % 2:
                        nc.vector.memset(mz[:], 0)
                    else:
                        nc.scalar.activation(
                            out=mz[:].bitcast(mybir.dt.bfloat16),
                            in_=zeros_t[:, 0:1].to_broadcast(
                                [128, max_sb_cols * EW // 2]),
                            func=mybir.ActivationFunctionType.Copy)

                idx_off = 0
                col_off = 0
                relu_flip = 0
                prev = None  # (comb_t, sbsz, k) pending phase B

                for k in range(NSB):
                    sbsz = SBS[k]
                    wbA, wbB = calls[k]
                    ncA, ncB = len(wbA), len(wbB)
                    ncols = ncA + ncB
                    for si in range(4 * k + 12, min(4 * k + 16, NSB * 4)):
                        nc.gpsimd.reg_load(vrs[si % NVR],
                                           vreg_t[0:1, si:si + 1])
                    m_t = msgsp.tile([128, max_sb_cols * EW], mdt,
                                     tag="msgs")
                    m3 = m_t[:].rearrange("p (c e) -> p c e", e=EW)
                    for (xsrc, c0, cn, ci) in (
                            (xab_lo, 0, ncA, 2 * k),
                            (xab_hi, ncA, ncB, 2 * k + 1)):
                        if cn == 0:
                            continue
                        nc2 = (cn + 1) // 2
                        for (sc0, scn, si) in ((c0, nc2, 2 * ci),
                                               (c0 + nc2, cn - nc2,
                                                2 * ci + 1)):
                            if scn == 0:
                                continue
                            nidx = scn * 128
                            nc.gpsimd.dma_gather(
                                out_ap=m3[:, sc0:sc0 + scn, :],
                                in_ap=xsrc[:],
                                idxs_ap=gidx_t[:, idx_off // 16:
                                               (idx_off + nidx) // 16],
                                num_idxs=nidx,
                                num_idxs_reg=vrs[si % 32],
                                elem_size=EW,
                                single_packet=False,
                                queue_num=gq,
                            )
                            gq = (gq + 1) % 4
                            idx_off += nidx

                    # bulk selection build: S[p, c, d] = (dstloc[p,c] == d)
                    s_t = spool.tile([128, max_sb_cols, WSZ], mdt,
                                     tag="sel")
                    try:
                        nc.vector.tensor_tensor(
                            out=s_t[:, 0:ncols, :],
                            in0=iota_t[:].unsqueeze(1).to_broadcast(
                                [128, ncols, WSZ]),
                            in1=dstloc_t[:, col_off:col_off + ncols]
                            .unsqueeze(2).to_broadcast([128, ncols, WSZ]),
                            op=mybir.AluOpType.is_equal,
                        )
                    except Exception:
                        for ci in range(ncols):
                            nc.vector.tensor_tensor(
                                out=s_t[:, ci, :],
                                in0=iota_t[:],
                                in1=dstloc_t[:, col_off + ci:col_off + ci + 1]
                                .to_broadcast([128, WSZ]),
                                op=mybir.AluOpType.is_equal,
                            )

                    agg_t = aggpsp.tile([128, 4, SB_FULL], f32, tag="agg")
                    wbs = list(wbA) + list(wbB)
                    for ci, wb in enumerate(wbs):
                        for fc in range(4):
                            nc.tensor.matmul(
                                out=agg_t[:, fc, wb:wb + WSZ],
                                lhsT=m3[:, ci, fc * 128:(fc + 1) * 128],
                                rhs=s_t[:, ci, :],
                                start=(ci == 0),
                                stop=(ci == len(wbs) - 1),
                            )

                    # comb assembly: agg x invdeg (DVE) + self-x rows (DMA)
                    comb_t = combp.tile([128, 8, SB_FULL], bf16, tag="comb")
                    sbb = SBB[k]
                    ivd = invdeg_t[:, sbb:sbb + sbsz]
                    comb4 = comb_t[:].rearrange(
                        "p (f two) d -> p f two d", two=2)
                    # even batches live in partitions 0:64 of agg, odd in
                    # 64:128; scale all 4 fc in one op per parity
                    nc.vector.tensor_mul(
                        out=comb4[0:64, :, 0, 0:sbsz],
                        in0=agg_t[0:64, :, 0:sbsz],
                        in1=ivd[0:64, :].unsqueeze(1)
                        .to_broadcast([64, 4, sbsz]))
                    nc.vector.tensor_mul(
                        out=comb4[64:128, :, 1, 0:sbsz],
                        in0=agg_t[64:128, :, 0:sbsz],
                        in1=ivd[64:128, :].unsqueeze(1)
                        .to_broadcast([64, 4, sbsz]))
                    # x of even batches -> partitions 64:128 of even slots
                    nc.sync.dma_start(
                        out=comb4[64:128, :, 0, 0:sbsz],
                        in_=xsT_ev[k, :, :, 0:sbsz],
                    )
                    # x of odd batches -> partitions 0:64 of odd slots
                    nc.sync.dma_start(
                        out=comb4[0:64, :, 1, 0:sbsz],
                        in_=xsT_od[k, :, :, 0:sbsz],
                    )

                    col_off += ncols

                    # phase B for the PREVIOUS superblock (software pipeline
                    # so PE has work while DVE drains agg of this one)
                    if prev is not None:
                        relu_flip = _phase_b(
                            nc, mybir, tc, hpsp, hsbp, out,
                            w_ev_t, w_od_t,
                            bias_t if has_bias else None,
                            prev, relu_flip)
                    prev = (comb_t, sbsz, k)

                relu_flip = _phase_b(
                    nc, mybir, tc, hpsp, hsbp, out, w_ev_t, w_od_t,
                    bias_t if has_bias else None, prev, relu_flip)

    nc.compile()
    names = dict(
        xab_lo=xab_lo.name, xab_hi=xab_hi.name,
        xsT_ev=xsT_ev.name, xsT_od=xsT_od.name,
        gidx=gidx.name, dstloc=dstloc.name,
        iota_rep=iota_rep.name, invdeg_rep=invdeg_rep.name,
        w_ev=w_ev.name, w_od=w_od.name, zeros_d=zeros_d.name,
        vreg=vreg_d.name, out=out.name,
        bias_d=(bias_d.name if has_bias else None),
    )
    return nc, names


def _phase_b(nc, mybir, tc, hpsp, hsbp, out, w_ev_t, w_od_t, bias_t,
             prev, relu_flip):
    f32 = mybir.dt.float32
    bf16 = mybir.dt.bfloat16
    comb_t, sbsz, k = prev
    sbb = SBB[k]
    hsb_t = hsbp.tile([128, 16, SB_FULL], bf16, tag="hsb")
    # hsb slot index = b*2 + hh = 4*fc + 2*par + hh
    hsb4 = hsb_t[:, :, 0:sbsz].rearrange(
        "p (b2 four) d -> p b2 four d", four=4)
    for hh in range(2):
        for par in range(2):
            w_t = w_od_t if par else w_ev_t
            for fc in range(4):
                b = 2 * fc + par
                pairpos = fc % 2
                if pairpos == 0:
                    h_ps = hpsp.tile([128, 2, SB_FULL], f32, tag="hps")
                nc.tensor.matmul(
                    out=h_ps[:, pairpos, 0:sbsz],
                    lhsT=w_t[:, hh * 128:(hh + 1) * 128],
                    rhs=comb_t[:, b, 0:sbsz],
                    start=True, stop=True,
                )
                if bias_t is not None:
                    nc.vector.tensor_add(
                        out=h_ps[:, pairpos, 0:sbsz],
                        in0=h_ps[:, pairpos, 0:sbsz],
                        in1=bias_t[:, hh:hh + 1].to_broadcast([128, sbsz]))
                if pairpos == 1:
                    # relu+cast both fc of the pair tile in one op
                    dst = hsb4[:, fc - 1:fc + 1, 2 * par + hh, :]
                    if relu_flip % 8 == 0:
                        nc.vector.tensor_relu(
                            out=dst, in_=h_ps[:, :, 0:sbsz])
                    else:
                        nc.scalar.activation(
                            out=dst, in_=h_ps[:, :, 0:sbsz],
                            func=mybir.ActivationFunctionType.Relu)
                    relu_flip += 1
    nc.sync.dma_start(
        out=out[k, :, :, :, 0:sbsz],
        in_=hsb_t[:, :, 0:sbsz].rearrange("p (b h) d -> p b h d", h=2),
    )
    return relu_flip


def _prep(x, edge_src, edge_dst, W_l, b_l, W_r):
    from ml_dtypes import bfloat16

    deg = np.bincount(edge_dst, minlength=N_NODES).astype(np.float32)
    invdeg = (1.0 / np.maximum(deg, 1.0)).astype(np.float32)
    invdeg[deg == 0] = 0.0  # mask never-written agg lanes

    if MSGS_FP8:
        from ml_dtypes import float8_e4m3 as mdt_np
    else:
        mdt_np = bfloat16
    xn = np.ascontiguousarray(x.transpose(1, 0, 2)).reshape(N_NODES, EW)
    xn_m = xn.astype(mdt_np)
    zrow = np.zeros((1, EW), dtype=mdt_np)
    xab_lo = np.ascontiguousarray(np.vstack([xn_m[:HALF], zrow]))
    xab_hi = np.ascontiguousarray(np.vstack([xn_m[HALF:], zrow]))

    iota_rep = np.broadcast_to(
        np.arange(WSZ, dtype=np.float32).astype(bfloat16)[None, :],
        (128, WSZ)).copy()
    WlT = W_l.T.astype(np.float32)
    WrT = W_r.T.astype(np.float32)
    w_ev = np.vstack([WlT, WrT]).astype(bfloat16)
    w_od = np.vstack([WrT, WlT]).astype(bfloat16)
    zeros_d = np.zeros((128, SB_FULL), dtype=bfloat16)
    has_bias = bool(np.any(b_l != 0))
    bias_d = (np.stack([b_l[:128], b_l[128:]], axis=1).astype(np.float32)
              if has_bias else None)

    core = edge_dst // ND
    # per (core, sb, half, window): sorted edge lists
    percore = []
    NW = [sz // WSZ for sz in SBS]  # windows per sb: 4,...,4,2
    cnt = np.zeros((NCORE, NSB, 2, 4), np.int64)
    for c in range(NCORE):
        sel = core == c
        ed = (edge_dst[sel] - c * ND).astype(np.int64)
        es = edge_src[sel].astype(np.int64)
        sb = np.minimum(ed // SB_FULL, NSB - 1)
        h = (es >= HALF).astype(np.int64)
        w = (ed - sb * SB_FULL) // WSZ
        order = np.lexsort((es, ed, w, h, sb))
        ed, es, sb, h, w = (a[order] for a in (ed, es, sb, h, w))
        key = ((sb * 2 + h) * 4 + w)
        bounds = np.searchsorted(key, np.arange(NSB * 2 * 4 + 1))
        for sbi in range(NSB):
            for hh in range(2):
                for wi in range(NW[sbi]):
                    kk = (sbi * 2 + hh) * 4 + wi
                    n = bounds[kk + 1] - bounds[kk]
                    cnt[c, sbi, hh, wi] = -(-n // 128)
        percore.append((ed, es, bounds))

    K = cnt.max(axis=0)  # [NSB, 2, 4] column slots per window

    # slot order per (sb, half): by cross-core usage count descending, so
    # each core's real edges cluster at the front of the call and the
    # trailing -1 trim (zero HBM cost) removes most SPMD padding
    winbase_lists = []
    slotmaps = []  # per (sb, half): list of (window, round)
    for sbi in range(NSB):
        for hh in range(2):
            slots = []
            maxr = int(K[sbi, hh, :NW[sbi]].max()) if NW[sbi] else 0
            for r in range(maxr):
                for wi in range(NW[sbi]):
                    if K[sbi, hh, wi] > r:
                        usage = int((cnt[:, sbi, hh, wi] > r).sum())
                        slots.append((-usage, r, wi))
            slots = [(wi, r) for _, r, wi in sorted(slots)]
            slotmaps.append(slots)
            winbase_lists.append(tuple(wi * WSZ for wi, r in slots))

    tot_cols = sum(len(s) for s in slotmaps)

    max_sb_cols = max(
        len(slotmaps[2 * k]) + len(slotmaps[2 * k + 1]) for k in range(NSB))

    # Build per-(call, core) slot index/dst arrays, then a SHARED valid
    # count V per call: num_idxs_reg is baked into the SPMD instruction so
    # every core must have exactly V non-negative indices (trailing -1s
    # are skipped by the gather ucode at zero HBM cost). A -1 slot exposes
    # stale SBUF, which is only safe if a previous superblock on the same
    # msgs buffer gathered that slot (coverage is uniform across cores
    # because all cores gather exactly [0, V)).
    call_sidx = [[None] * NCORE for _ in range(NSB * 2)]
    for c in range(NCORE):
        ed, es, bounds = percore[c]
        for sbi in range(NSB):
            for hh in range(2):
                slots = slotmaps[sbi * 2 + hh]
                nslots = len(slots)
                sidx = np.full((nslots, 128), -1, np.int32)
                for si, (wi, r) in enumerate(slots):
                    kk = (sbi * 2 + hh) * 4 + wi
                    lo, hi = bounds[kk], bounds[kk + 1]
                    e0 = lo + r * 128
                    e1 = min(lo + (r + 1) * 128, hi)
                    if e1 > e0:
                        n = e1 - e0
                        sidx[si, :n] = es[e0:e1] - hh * HALF
                call_sidx[sbi * 2 + hh][c] = sidx

    # per-core dstloc (separate pass, simple)
    dl_cores = np.full((NCORE, tot_cols, 128), -1.0, np.float32)
    for c in range(NCORE):
        ed, es, bounds = percore[c]
        ci_base = 0
        for sbi in range(NSB):
            for hh in range(2):
                slots = slotmaps[sbi * 2 + hh]
                for si, (wi, r) in enumerate(slots):
                    kk = (sbi * 2 + hh) * 4 + wi
                    lo, hi = bounds[kk], bounds[kk + 1]
                    e0 = lo + r * 128
                    e1 = min(lo + (r + 1) * 128, hi)
                    if e1 > e0:
                        dl_cores[c, ci_base + si, :e1 - e0] = (
                            ed[e0:e1] - SBB[sbi] - wi * WSZ)
                ci_base += len(slots)

    # per-core valid counts (num_idxs_reg loaded from data at runtime).
    # Each (sb, half) call is split in two sub-calls on different SWDGE
    # queues so the latency-bound random reads drain through more rings.
    # The msgs buffers are memset to zero once at kernel start, so slots
    # beyond the valid regions expose only finite stale data (S=0 kills).
    schedule = tuple(winbase_lists)
    Vs = np.zeros((NCORE, NSB * 4), np.int32)
    for c in range(NCORE):
        for ci in range(NSB * 2):
            nslots = len(slotmaps[ci])
            nidx = nslots * 128
            if nidx == 0:
                continue
            nc2 = (nslots + 1) // 2
            flat = call_sidx[ci][c].reshape(-1)
            nz = np.nonzero(flat >= 0)[0]
            V = int(nz[-1]) + 1 if len(nz) else 0
            V = min(max(V, 16), nidx)
            if nc2 == nslots:
                Vs[c, 2 * ci] = V
            else:
                Vs[c, 2 * ci] = min(V, nc2 * 128)
                Vs[c, 2 * ci + 1] = min(max(V - nc2 * 128, 16),
                                        (nslots - nc2) * 128)

    in_maps = []
    for c in range(NCORE):
        gidx_chunks = []
        for ci in range(NSB * 2):
            nslots = len(slotmaps[ci])
            if nslots == 0:
                continue
            nc2 = (nslots + 1) // 2
            flat = call_sidx[ci][c].reshape(-1).copy()
            valid = np.zeros(nslots * 128, bool)
            valid[:Vs[c, 2 * ci]] = True
            if nc2 < nslots:
                valid[nc2 * 128:nc2 * 128 + Vs[c, 2 * ci + 1]] = True
            flat[valid & (flat < 0)] = ZROW
            flat[~valid] = -1
            sl = flat.astype(np.int16)
            gidx_chunks.append(np.tile(sl.reshape(-1, 16).T, (8, 1)))
        gidx_arr = np.ascontiguousarray(np.concatenate(gidx_chunks, axis=1))
        dstloc = np.ascontiguousarray(dl_cores[c].T.astype(bfloat16))
        vreg_c = np.broadcast_to(Vs[c][None, :], (128, NSB * 4)).copy()

        invdeg_c = np.broadcast_to(
            invdeg[c * ND:(c + 1) * ND].astype(bfloat16)[None, :],
            (128, ND)).copy()
        xc = x[:, c * ND:(c + 1) * ND, :]  # [8, ND, 64]
        # [NSB, F_IN, 4, SB_FULL] superblock-major, zero-padded last sb
        xsT_ev = np.zeros((NSB, F_IN, 4, SB_FULL), bfloat16)
        xsT_od = np.zeros((NSB, F_IN, 4, SB_FULL), bfloat16)
        for k in range(NSB):
            sl = xc[:, SBB[k]:SBB[k] + SBS[k], :]  # [8, sbsz, 64]
            xsT_ev[k, :, :, :SBS[k]] = sl[0::2].transpose(2, 0, 1)
            xsT_od[k, :, :, :SBS[k]] = sl[1::2].transpose(2, 0, 1)

        in_maps.append(dict(
            xab_lo=xab_lo, xab_hi=xab_hi, xsT_ev=xsT_ev, xsT_od=xsT_od,
            gidx=gidx_arr, dstloc=dstloc, iota_rep=iota_rep,
            invdeg_rep=invdeg_c, w_ev=w_ev, w_od=w_od, zeros_d=zeros_d,
            vreg=vreg_c, bias_d=bias_d,
        ))
    return schedule, has_bias, in_maps


def kernel(x, edge_src, edge_dst, W_l, b_l, W_r):
    from concourse.bass_utils import run_bass_kernel_spmd

    x = np.asarray(x, dtype=np.float32)
    edge_src = np.asarray(edge_src, dtype=np.int32)
    edge_dst = np.asarray(edge_dst, dtype=np.int32)
    W_l = np.asarray(W_l, dtype=np.float32)
    b_l = np.asarray(b_l, dtype=np.float32)
    W_r = np.asarray(W_r, dtype=np.float32)

    schedule, has_bias, in_maps = _prep(x, edge_src, edge_dst, W_l, b_l, W_r)
    key = (schedule, has_bias)
    if key not in _cache:
        _cache[key] = _build(schedule, has_bias)
    nc, names = _cache[key]

    run_maps = []
    for m in in_maps:
        rm = {names[k]: v for k, v in m.items()
              if names.get(k) is not None and v is not None}
        run_maps.append(rm)
    res = run_bass_kernel_spmd(nc, run_maps, list(range(NCORE)))
    outs = []
    for c in range(NCORE):
        oc = np.asarray(res.results[c][names["out"]]).astype(np.float32)
        # [NSB, 128 hid, 8, 2, SB_FULL] -> [8, ND, 256]
        full = oc.transpose(2, 0, 4, 3, 1).reshape(
            BATCH, NSB * SB_FULL, F_HID)
        parts = [full[:, k * SB_FULL:k * SB_FULL + SBS[k]]
                 for k in range(NSB)]
        outs.append(np.concatenate(parts, axis=1))
    return np.concatenate(outs, axis=1)
